# revision 75
# baseline (speedup 1.0000x reference)
"""Sliding-window causal GQA attention (RoPE) for Trainium2, 8-core SPMD.

Problem: x:(4,2048,2048), Wq:(2048,2048), Wk/Wv:(512,2048), Wo:(2048,2048)
  q = rope(x @ Wq.T) 16 heads, k/v = (x @ Wk.T / x @ Wv.T) 4 kv heads (GQA x4),
  causal sliding-window attention (W=1024), out = z @ Wo.T.

Sharding: 8 cores = 4 batches x 2 head-groups (8 q-heads / 2 kv-heads each).
Each core computes a partial output (its head-group's Wo contribution) for its
batch; host sums the two partials per batch.

Per-core kernel (projections/scores in f32r; exp'd probabilities bf16):
  - layout: qT/kT as (head_dim, L) ["transposed"], v as (L, head_dim)
  - scores computed transposed S.T (keys on partitions, queries free) so P.T
    feeds the PV matmul directly with no on-chip transposes.
  - softmax denominator: ones[128,128] stationary matmul accumulates the
    per-query sum broadcast across all 128 partitions directly in PSUM
    (no separate M=1 sum + K=1 broadcast matmuls).
  - no max-subtraction in softmax: logits are O(1) here, exp is safe.
  - sliding window at 128-block granularity: query-super of 256 x up to 10
    key-blocks; boundary blocks masked via precomputed 0/1 tiles.
  - lag-2 software pipeline: the denominator/PV of super t are issued two
    score-slots later, so the PE never waits on exp/mask/RoPE latency.
  - inputs are host-prepacked so each DMA moves long contiguous runs per
    partition, keeping DMA packet counts low.
"""

import math
import numpy as np

H = 16
D = 4
WINDOW = 1024
THETA = 10000.0
N, L, E = 4, 2048, 2048
P = 128
DH = E // H          # 128 head dim
NH = H // 2          # 8 q heads per core
NKV = 2              # kv heads per core
NB = L // P          # 16 key blocks
NKT = E // P         # 16 contraction tiles
SCALE = 1.0 / math.sqrt(DH)

_NC = None


def _kbs_for_super(t):
    """Key blocks overlapping the window of query super t (256 queries)."""
    return list(range(max(0, 2 * t - 8), 2 * t + 2))


def build_nc():
    from contextlib import ExitStack
    from concourse import bacc, tile, mybir

    F32 = mybir.dt.float32
    F32R = mybir.dt.float32r
    BF16 = mybir.dt.bfloat16
    EXP = mybir.ActivationFunctionType.Exp

    SHUF_SWAP = [i ^ 1 for i in range(32)]

    nc = bacc.Bacc("TRN2", target_bir_lowering=False, debug=False)
    # prepacked inputs (see _pack_core_inputs for layouts); x and the
    # projection weights ship bf16 (halves DMA; softmax cancels most of
    # the logit-path quantization noise)
    xq = nc.dram_tensor("xq", [4 * P, NKT * 512], BF16, kind="ExternalInput").ap()
    wqp = nc.dram_tensor("wqp", [NH * P, NKT * DH], BF16, kind="ExternalInput").ap()
    wkv = nc.dram_tensor("wkv", [P, NKT * 512], BF16, kind="ExternalInput").ap()
    woT = nc.dram_tensor("woT", [NH * DH, E], BF16, kind="ExternalInput").ap()
    cosT = nc.dram_tensor("cosT", [P, L], F32, kind="ExternalInput").ap()
    sinT = nc.dram_tensor("sinT", [P, L], F32, kind="ExternalInput").ap()
    U8 = mybir.dt.uint8
    masks = nc.dram_tensor("masks", [2 * P, 512], U8, kind="ExternalInput").ap()
    out = nc.dram_tensor("out", [L, E], BF16, kind="ExternalOutput").ap()
    zspill = nc.dram_tensor("zspill", [NH * P, L], BF16).ap()

    with tile.TileContext(nc) as tc, ExitStack() as stk:
        resid = stk.enter_context(tc.tile_pool(name="resid", bufs=1))
        kT = [resid.tile([P, L], BF16, tag=f"kT{i}", name=f"kT{i}") for i in range(NKV)]
        kvk = [resid.tile([P, 1024], BF16, tag=f"kvk{dc}", name=f"kvk{dc}")
               for dc in range(4)]
        kvv = [resid.tile([P, 1024], BF16, tag=f"kvv{dc}", name=f"kvv{dc}")
               for dc in range(4)]
        vt = [[resid.tile([P, P], BF16, tag=f"v{i}_{b}", name=f"v{i}_{b}") for b in range(NB)]
              for i in range(NKV)]

        z3 = [resid.tile([P, 512], BF16, tag=f"z3_{h}", name=f"z3_{h}")
              for h in range(NH)]
        # wo staging (bf16); loads issued near the end of attention so the
        # output projection never waits on them
        wostg = stk.enter_context(tc.tile_pool(name="wostg", bufs=1))
        stg = [wostg.tile([P, E], BF16, tag=f"wostg{h}", name=f"stg{h}")
               for h in range(NH)]
        zinbpool = stk.enter_context(tc.tile_pool(name="zinb", bufs=16))

        def load_z(qsb):
            if qsb == 3:
                return z3
            zin = []
            for h in range(NH):
                zb = zinbpool.tile([P, 512], BF16, tag="zinb")
                nc.sync.dma_start(
                    out=zb[:],
                    in_=zspill[h * P:(h + 1) * P, qsb * 512:(qsb + 1) * 512])
                zin.append(zb)
            return zin
        const = stk.enter_context(tc.tile_pool(name="const", bufs=1))
        # INVERTED combined boundary masks (1 = disallowed), each covering a
        # 2-keyblock pair; applied via copy_predicated(pt, mask, zeros) so
        # exp'd garbage from skipped PSUM regions (possibly inf/NaN) is
        # overwritten with exact zeros rather than multiplied.
        mdiag = const.tile([P, 512], U8, tag="mdiag")
        mfar = const.tile([P, 512], U8, tag="mfar")
        zeros = const.tile([P, 512], BF16, tag="zeros")
        ones_f = const.tile([P, P], F32, tag="ones_f")
        ones = const.tile([P, P], BF16, tag="ones")

        def rope_evict(dest, psum, cos_sl, sin_sl, tmp_pool, n):
            # dest = psum * cos + pairswap(psum) * sin   (sin pre-signed)
            tmp = tmp_pool.tile([P, 512], F32, tag="ropetmp", name="ropetmp")
            nc.vector.stream_shuffle(tmp[:, :n], psum, SHUF_SWAP)
            nc.vector.tensor_mul(tmp[:, :n], tmp[:, :n], sin_sl)
            nc.vector.tensor_mul(dest, psum, cos_sl)
            nc.vector.tensor_add(dest, dest, tmp[:, :n])

        osb = stk.enter_context(tc.tile_pool(name="osb", bufs=8))
        with tc.tile_pool(name="pacc", bufs=5, space="PSUM") as pacc, \
             tc.tile_pool(name="pz", bufs=2, space="PSUM") as pzp, \
             tc.tile_pool(name="pb", bufs=1, space="PSUM") as pbp, \
             tc.tile_pool(name="quarter", bufs=2) as qpool, \
             tc.tile_pool(name="wq", bufs=3) as wqpool, \
             tc.tile_pool(name="work", bufs=4) as work, \
             tc.tile_pool(name="qt", bufs=2) as qtpool, \
             tc.tile_pool(name="zev", bufs=3) as zevpool, \
             tc.tile_pool(name="scr", bufs=2) as scrpool, \
             tc.tile_pool(name="rtmp", bufs=2) as rtmp:

            def load_quarter(qtr):
                xtc = [qpool.tile([P, 2048], BF16, tag=f"xt{dc}", name=f"xt{dc}")
                       for dc in range(4)]
                cos_q = qpool.tile([P, 512], F32, tag="cos")
                sin_q = qpool.tile([P, 512], F32, tag="sin")
                for dc in range(4):
                    nc.sync.dma_start(
                        out=xtc[dc][:],
                        in_=xq[qtr * P:(qtr + 1) * P, dc * 2048:(dc + 1) * 2048])
                c0 = 512 * qtr
                nc.sync.dma_start(out=cos_q[:], in_=cosT[:, c0:c0 + 512])
                nc.sync.dma_start(out=sin_q[:], in_=sinT[:, c0:c0 + 512])
                return xtc, cos_q, sin_q

            # startup: interleave kvw/x chunks in consumption order so the
            # first K-proj matmuls start after ~2MB instead of ~8MB
            xtc0 = [qpool.tile([P, 2048], BF16, tag=f"xt{dc}", name=f"xt{dc}")
                    for dc in range(4)]
            cos_q0 = qpool.tile([P, 512], F32, tag="cos")
            sin_q0 = qpool.tile([P, 512], F32, tag="sin")
            # K-half weights + x chunks first (the only startup-critical 3MB);
            # V-half weights and tables follow
            for dc in range(4):
                nc.sync.dma_start(out=kvk[dc][:],
                                  in_=wkv[:, dc * 1024:(dc + 1) * 1024])
                nc.sync.dma_start(out=xtc0[dc][:],
                                  in_=xq[:P, dc * 2048:(dc + 1) * 2048])
                if dc == 0:
                    nc.sync.dma_start(out=cos_q0[:], in_=cosT[:, :512])
                    nc.sync.dma_start(out=sin_q0[:], in_=sinT[:, :512])
            for dc in range(4):
                nc.sync.dma_start(out=kvv[dc][:],
                                  in_=wkv[:, 4096 + dc * 1024:4096 + (dc + 1) * 1024])
            cur = (xtc0, cos_q0, sin_q0)

            # rolling Wq prefetch, 3 heads deep (global head index)
            wq_q = []

            def wq_prefetch(g):
                if g >= 4 * NH:
                    return
                h = g % NH
                wqt = wqpool.tile([P, NKT * DH], BF16, tag="wqh", name="wqt")
                nc.sync.dma_start(out=wqt[:], in_=wqp[h * P:(h + 1) * P, :])
                wq_q.append(wqt)

            nc.sync.dma_start(out=mdiag[:], in_=masks[0:P, :])
            nc.sync.dma_start(out=mfar[:], in_=masks[P:2 * P, :])
            nc.vector.memset(ones_f[:], 1.0)
            nc.vector.tensor_copy(ones[:], ones_f[:])
            nc.vector.memset(zeros[:], 0.0)

            # lag-2 pipeline of attention tails
            pend = []

            def denom_tree(pt, nkb):
                # softmax denominator part 1: fold the key-block dim on DVE
                # (tree of contiguous pairwise adds; dead boundary halves are
                # zeroed by the masks). Issued right after the masks so the
                # DVE work is long done when the tail's pb matmul needs it.
                ksum = zevpool.tile([P, 256], BF16, tag="ksum")

                def fl(tile_, a, b):
                    return tile_[:, a:b, :].rearrange("p k q -> p (k q)")

                if nkb == 2:
                    nc.vector.tensor_add(ksum[:], pt[:, 0, :], pt[:, 1, :])
                    return ksum
                scr = scrpool.tile([P, 5, 256], BF16, tag="scr")
                hb = nkb // 2
                nc.vector.tensor_add(fl(scr, 0, hb), fl(pt, 0, hb), fl(pt, hb, nkb))
                if nkb == 4:
                    nc.vector.tensor_add(ksum[:], scr[:, 0, :], scr[:, 1, :])
                elif nkb == 6:
                    nc.vector.tensor_add(scr[:, 0, :], scr[:, 0, :], scr[:, 1, :])
                    nc.vector.tensor_add(ksum[:], scr[:, 0, :], scr[:, 2, :])
                elif nkb == 8:
                    nc.vector.tensor_add(fl(scr, 0, 2), fl(scr, 0, 2), fl(scr, 2, 4))
                    nc.vector.tensor_add(ksum[:], scr[:, 0, :], scr[:, 1, :])
                else:  # nkb == 10
                    nc.vector.tensor_add(fl(scr, 0, 2), fl(scr, 0, 2), fl(scr, 2, 4))
                    nc.vector.tensor_add(scr[:, 0, :], scr[:, 0, :], scr[:, 1, :])
                    nc.vector.tensor_add(ksum[:], scr[:, 0, :], scr[:, 4, :])
                return ksum

            def attn_tail():
                kv, kbs, pt, h, t, ksum = pend.pop(0)
                nkb = len(kbs)

                def qspan(kb):
                    # diagB (kb==2t+1) is all-masked for the first 128 queries
                    # of the super; farA (kb==2t-8) for the last 128. Skip the
                    # dead half in the PV matmuls (partial-width PSUM
                    # accumulation zero-fills untouched columns).
                    if kb == 2 * t + 1:
                        return 128, 256
                    if kb == 2 * t - 8:
                        return 0, 128
                    return 0, 256

                # PV
                pz = pzp.tile([P, 256], F32, tag="pz")
                for i, kb in enumerate(kbs):
                    a, b = qspan(kb)
                    nc.tensor.matmul(
                        pz[:, a:b], vt[kv][kb][:],
                        pt[:, i, a:b],
                        start=(i == 0), stop=(i == nkb - 1))
                # denominator part 2: one ones-matmul broadcasts the
                # cross-partition sum into PSUM
                pb = pbp.tile([P, 256], F32, tag="pb")
                nc.tensor.matmul(pb[:], ones[:], ksum[:], start=True, stop=True)
                rec = zevpool.tile([P, 256], F32, tag="rec")
                nc.vector.reciprocal_approx_fast(rec[:], pb[:])
                if t >= 6:
                    s3 = t - 6
                    nc.vector.tensor_mul(
                        z3[h][:, s3 * 256:(s3 + 1) * 256], pz[:], rec[:])
                else:
                    zev = zevpool.tile([P, 256], BF16, tag="zev")
                    nc.vector.tensor_mul(zev[:], pz[:], rec[:])
                    nc.sync.dma_start(
                        out=zspill[h * P:(h + 1) * P, t * 256:(t + 1) * 256],
                        in_=zev[:])

            for qtr in range(4):
                xt, cos_q, sin_q = cur

                def xtile(kt, a, b):
                    return xt[kt // 4][:, (kt % 4) * 512 + a: (kt % 4) * 512 + b]

                def kvw_sl(kt, a, b):
                    # K-half (a,b in [0,256)) from kvk, V-half from kvv
                    if b <= 256:
                        return kvk[kt // 4][:, (kt % 4) * 256 + a: (kt % 4) * 256 + b]
                    return kvv[kt // 4][:, (kt % 4) * 256 + a - 256: (kt % 4) * 256 + b - 256]

                c0 = 512 * qtr

                # K projection (+RoPE) for both kv heads
                for kv in range(NKV):
                    pk = pacc.tile([P, 512], F32, tag="pacc")
                    for kt in range(NKT):
                        nc.tensor.matmul(
                            pk[:],
                            kvw_sl(kt, kv * DH, (kv + 1) * DH),
                            xtile(kt, 0, 512),
                            start=(kt == 0), stop=(kt == NKT - 1),
                        )
                    rope_evict(kT[kv][:, c0:c0 + 512], pk[:], cos_q[:], sin_q[:], rtmp, 512)

                if qtr == 0:
                    # deferred so these don't share DMA bandwidth with the
                    # critical kvw+xt startup burst
                    for g0 in range(3):
                        wq_prefetch(g0)

                # V projection (both kv heads at once, natural layout)
                for lb in range(4):
                    pv = pacc.tile([P, 512], F32, tag="pacc")
                    for kt in range(NKT):
                        nc.tensor.matmul(
                            pv[:, :NKV * DH],
                            xtile(kt, lb * P, (lb + 1) * P),
                            kvw_sl(kt, 256, 512),
                            start=(kt == 0), stop=(kt == NKT - 1),
                        )
                    for kv in range(NKV):
                        nc.scalar.copy(vt[kv][4 * qtr + lb][:], pv[:, kv * DH:(kv + 1) * DH])

                def q_proj(g):
                    """Q projection + RoPE for one head; returns the roped tile."""
                    wq = wq_q.pop(0)
                    wq_prefetch(g + 3)
                    pq = pacc.tile([P, 512], F32, tag="pacc")
                    for kt in range(NKT):
                        nc.tensor.matmul(
                            pq[:],
                            wq[:, kt * DH:(kt + 1) * DH],
                            xtile(kt, 0, 512),
                            start=(kt == 0), stop=(kt == NKT - 1),
                        )
                    qth = qtpool.tile([P, 512], BF16, tag="qt")
                    rope_evict(qth[:], pq[:], cos_q[:], sin_q[:], rtmp, 512)
                    return qth

                # Q projection + attention, head-major; Q runs one head ahead
                # so RoPE latency is always covered by PE work
                qnext = q_proj(qtr * NH)
                for h in range(NH):
                    kv = h // (NH // NKV)
                    qth = qnext
                    if h + 1 < NH:
                        qnext = q_proj(qtr * NH + h + 1)
                    if h == 1 and qtr < 3:
                        cur = load_quarter(qtr + 1)
                    if h == 1 and qtr == 3:
                        # prefetch wo for the output projection; by now the
                        # DMA engines are mostly idle
                        for hh in range(NH):
                            nc.sync.dma_start(out=stg[hh][:],
                                              in_=woT[hh * P:(hh + 1) * P, :])
                    if h == 4 and qtr == 3:
                        zpre0 = load_z(0)
                    if h == 6 and qtr == 3:
                        zpre1 = load_z(1)
                    for s in range(2):
                        t = 2 * qtr + s
                        qt = qth[:, s * 256:(s + 1) * 256]

                        # drain the oldest pending super (lag 2)
                        if len(pend) >= 2:
                            attn_tail()

                        kbs = _kbs_for_super(t)
                        nkb = len(kbs)
                        pt = work.tile([P, 10, 256], BF16, tag="pt")
                        # scores (transposed: keys on partitions) in chunks of
                        # 2 kb; dead query-halves of the boundary blocks are
                        # skipped (untouched PSUM reads back as zero, and the
                        # full-width masks zero those slots after exp)
                        for ci in range(0, nkb, 2):
                            cn = min(2, nkb - ci)
                            ps = pacc.tile([P, 512], F32, tag="pacc")
                            for i in range(cn):
                                kb = kbs[ci + i]
                                if kb == 2 * t + 1:
                                    a, b = 128, 256
                                elif kb == 2 * t - 8:
                                    a, b = 0, 128
                                else:
                                    a, b = 0, 256
                                nc.tensor.matmul(
                                    ps[:, i * 256 + a:i * 256 + b],
                                    kT[kv][:, kb * P:(kb + 1) * P],
                                    qt[:, a:b],
                                    start=True, stop=True,
                                )
                            nc.scalar.activation(
                                pt[:, ci:ci + cn, :].rearrange("p k q -> p (k q)"),
                                ps[:, :cn * 256], EXP, scale=SCALE)
                        # combined window masks on the boundary pairs (the
                        # diag pair is always the last two blocks; the far
                        # pair exists only for t>=4 and is the first two)
                        dsl = pt[:, nkb - 2:nkb, :].rearrange("p k q -> p (k q)")
                        nc.vector.copy_predicated(dsl, mdiag[:], zeros[:])
                        if t >= 4:
                            fsl = pt[:, 0:2, :].rearrange("p k q -> p (k q)")
                            nc.vector.copy_predicated(fsl, mfar[:], zeros[:])
                        ksum = denom_tree(pt, nkb)
                        pend.append((kv, kbs, pt, h, t, ksum))

            while pend:
                attn_tail()

        # Output projection: out[q,:] += sum_h zTn_h[:,q].T @ woT[h]
        # wo and z are bf16 and feed the matmuls directly (bf16 matmul runs
        # at the same rate as f32r; the data was bf16-quantized anyway).
        with tc.tile_pool(name="po", bufs=8, space="PSUM") as pop:
            zs = {0: zpre0, 1: zpre1}
            for qsb in range(4):
                zin = zs.pop(qsb) if qsb in zs else load_z(qsb)
                for ec in range(4):
                    po = [pop.tile([P, 512], F32, tag="po", name=f"po{i}")
                          for i in range(4)]
                    for h in range(NH):
                        for qb in range(4):
                            nc.tensor.matmul(
                                po[qb][:],
                                zin[h][:, qb * P:(qb + 1) * P],
                                stg[h][:, ec * 512:(ec + 1) * 512],
                                start=(h == 0), stop=(h == NH - 1),
                            )
                    for qb in range(4):
                        ot = osb.tile([P, 512], BF16, tag="ot")
                        # alternate copy engine so the final evictions drain 2x
                        if qb % 2 == 0:
                            nc.scalar.copy(ot[:], po[qb][:])
                        else:
                            nc.vector.tensor_copy(ot[:], po[qb][:])
                        nc.sync.dma_start(
                            out=out[qsb * 512 + qb * P: qsb * 512 + (qb + 1) * P,
                                    ec * 512:(ec + 1) * 512],
                            in_=ot[:])
                    if qsb == 0 and ec == 3:
                        # prefetch qsb2 (reuses qsb0's buffers; issued after
                        # all qsb0 matmuls so the WAR ordering is clean)
                        zs[2] = load_z(2)

    nc.compile()
    return nc


def _host_tables():
    freqs = 1.0 / (THETA ** (np.arange(0, DH - 1, 2, dtype=np.float64) / DH))
    ang = np.arange(L, dtype=np.float64)[:, None] * freqs[None, :]  # (L, 64)
    cos = np.cos(ang)
    sin = np.sin(ang)
    cosT = np.empty((P, L), np.float32)
    sinT = np.empty((P, L), np.float32)
    cosT[0::2, :] = cos.T
    cosT[1::2, :] = cos.T
    sinT[0::2, :] = -sin.T
    sinT[1::2, :] = sin.T
    return cosT, sinT


def _host_masks():
    # INVERTED masks: 1 where the slot is outside the attention window
    k = np.arange(P)[:, None]
    q = np.arange(256)[None, :]
    import ml_dtypes
    mdiag = np.concatenate([(k > q), (k > q - 128)], axis=1)     # (128, 512)
    mfar = np.concatenate([(k < q + 1), (k < q - 127)], axis=1)
    return np.concatenate([mdiag, mfar], axis=0).astype(np.uint8)


def _pack_core_inputs(x, Wq, Wk, Wv, Wo, n, g):
    """Prepacked per-core inputs; long contiguous per-partition DMA runs."""
    xT = np.ascontiguousarray(x[n].T)                      # (E, L)
    # xq[qtr*128+p, kt*512+c] = xT[kt*128+p, qtr*512+c]
    xq = xT.reshape(NKT, P, 4, 512).transpose(2, 1, 0, 3).reshape(4 * P, NKT * 512)
    # wqp[h*128+p, kt*128+c] = Wq.T[kt*128+p, g*1024+h*128+c]
    wqT = Wq[g * 1024:(g + 1) * 1024, :].T                 # (E, 1024)
    wqp = wqT.reshape(NKT, P, NH, DH).transpose(2, 1, 0, 3).reshape(NH * P, NKT * DH)
    xq = np.ascontiguousarray(xq)
    wqp = np.ascontiguousarray(wqp)
    # wkv[p, kt*512+j]: j<256 -> Wk.T slice, j>=256 -> Wv.T slice
    wkT = Wk[g * 256:(g + 1) * 256, :].T.reshape(NKT, P, 256)
    wvT = Wv[g * 256:(g + 1) * 256, :].T.reshape(NKT, P, 256)
    # K-half first, V-half second (V is not startup-critical)
    wkvp = np.concatenate([
        wkT.transpose(1, 0, 2).reshape(P, NKT * 256),
        wvT.transpose(1, 0, 2).reshape(P, NKT * 256)], axis=1)
    woT = Wo[:, g * 1024:(g + 1) * 1024].T                 # (1024, E)
    import ml_dtypes
    return {
        "xq": xq.astype(ml_dtypes.bfloat16),
        "wqp": wqp.astype(ml_dtypes.bfloat16),
        "wkv": np.ascontiguousarray(wkvp).astype(ml_dtypes.bfloat16),
        "woT": np.ascontiguousarray(woT).astype(ml_dtypes.bfloat16),
    }


def kernel(x, Wq, Wk, Wv, Wo):
    global _NC
    x = np.asarray(x, np.float32)
    Wq = np.asarray(Wq, np.float32)
    Wk = np.asarray(Wk, np.float32)
    Wv = np.asarray(Wv, np.float32)
    Wo = np.asarray(Wo, np.float32)

    if _NC is None:
        _NC = build_nc()
    nc = _NC

    cosT, sinT = _host_tables()
    masks = _host_masks()
    in_maps = []
    for c in range(8):
        n, g = c % 4, c // 4
        m = _pack_core_inputs(x, Wq, Wk, Wv, Wo, n, g)
        m.update({"cosT": cosT, "sinT": sinT, "masks": masks})
        in_maps.append(m)

    from concourse.bass_utils import run_bass_kernel_spmd
    res = run_bass_kernel_spmd(nc, in_maps, list(range(8)), trace=False)
    out = np.empty((N, L, E), np.float32)
    for n_ in range(4):
        out[n_] = (res.results[n_]["out"].astype(np.float32)
                   + res.results[4 + n_]["out"].astype(np.float32))
    return out


if __name__ == "__main__":
    rng = np.random.default_rng(0)
    x = rng.standard_normal((N, L, E), dtype=np.float32)
    Wq = (rng.standard_normal((E, E), dtype=np.float32) * 0.02)
    Wk = (rng.standard_normal((E // D, E), dtype=np.float32) * 0.02)
    Wv = (rng.standard_normal((E // D, E), dtype=np.float32) * 0.02)
    Wo = (rng.standard_normal((E, E), dtype=np.float32) * 0.02)
    print(kernel(x, Wq, Wk, Wv, Wo).shape)



# revision 80
# speedup vs baseline: 1.0330x; 1.0330x over previous
"""Sliding-window causal GQA attention (RoPE) for Trainium2, 8-core SPMD.

Problem: x:(4,2048,2048), Wq:(2048,2048), Wk/Wv:(512,2048), Wo:(2048,2048)
  q = rope(x @ Wq.T) 16 heads, k/v = (x @ Wk.T / x @ Wv.T) 4 kv heads (GQA x4),
  causal sliding-window attention (W=1024), out = z @ Wo.T.

Sharding: 8 cores = 4 batches x 2 head-groups (8 q-heads / 2 kv-heads each).
Each core computes a partial output (its head-group's Wo contribution) for its
batch; host sums the two partials per batch.

Per-core kernel (projections/scores in f32r; exp'd probabilities bf16):
  - layout: qT/kT as (head_dim, L) ["transposed"], v as (L, head_dim)
  - scores computed transposed S.T (keys on partitions, queries free) so P.T
    feeds the PV matmul directly with no on-chip transposes.
  - softmax denominator: ones[128,128] stationary matmul accumulates the
    per-query sum broadcast across all 128 partitions directly in PSUM
    (no separate M=1 sum + K=1 broadcast matmuls).
  - no max-subtraction in softmax: logits are O(1) here, exp is safe.
  - sliding window at 128-block granularity: query-super of 256 x up to 10
    key-blocks; boundary blocks masked via precomputed 0/1 tiles.
  - lag-2 software pipeline: the denominator/PV of super t are issued two
    score-slots later, so the PE never waits on exp/mask/RoPE latency.
  - inputs are host-prepacked so each DMA moves long contiguous runs per
    partition, keeping DMA packet counts low.
"""

import math
import numpy as np

H = 16
D = 4
WINDOW = 1024
THETA = 10000.0
N, L, E = 4, 2048, 2048
P = 128
DH = E // H          # 128 head dim
NH = H // 2          # 8 q heads per core
NKV = 2              # kv heads per core
NB = L // P          # 16 key blocks
NKT = E // P         # 16 contraction tiles
SCALE = 1.0 / math.sqrt(DH)

_NC = None


def _kbs_for_super(t):
    """Key blocks overlapping the window of query super t (256 queries)."""
    return list(range(max(0, 2 * t - 8), 2 * t + 2))


def build_nc():
    from contextlib import ExitStack
    from concourse import bacc, tile, mybir

    F32 = mybir.dt.float32
    F32R = mybir.dt.float32r
    BF16 = mybir.dt.bfloat16
    EXP = mybir.ActivationFunctionType.Exp

    SHUF_SWAP = [i ^ 1 for i in range(32)]

    nc = bacc.Bacc("TRN2", target_bir_lowering=False, debug=False)
    # prepacked inputs (see _pack_core_inputs for layouts); x and the
    # projection weights ship bf16 (halves DMA; softmax cancels most of
    # the logit-path quantization noise)
    xq = nc.dram_tensor("xq", [4 * P, NKT * 512], BF16, kind="ExternalInput").ap()
    wqp = nc.dram_tensor("wqp", [NH * P, NKT * DH], BF16, kind="ExternalInput").ap()
    wkv = nc.dram_tensor("wkv", [P, NKT * 512], BF16, kind="ExternalInput").ap()
    woT = nc.dram_tensor("woT", [NH * DH, E], BF16, kind="ExternalInput").ap()
    cosT = nc.dram_tensor("cosT", [P, L], F32, kind="ExternalInput").ap()
    sinT = nc.dram_tensor("sinT", [P, L], F32, kind="ExternalInput").ap()
    masks = nc.dram_tensor("masks", [2 * P, 512], BF16, kind="ExternalInput").ap()
    out = nc.dram_tensor("out", [L, E], BF16, kind="ExternalOutput").ap()
    zspill = nc.dram_tensor("zspill", [NH * P, L], BF16).ap()

    with tile.TileContext(nc) as tc, ExitStack() as stk:
        resid = stk.enter_context(tc.tile_pool(name="resid", bufs=1))
        kT = [resid.tile([P, L], BF16, tag=f"kT{i}", name=f"kT{i}") for i in range(NKV)]
        kvk = [resid.tile([P, 1024], BF16, tag=f"kvk{dc}", name=f"kvk{dc}")
               for dc in range(4)]
        kvv = [resid.tile([P, 1024], BF16, tag=f"kvv{dc}", name=f"kvv{dc}")
               for dc in range(4)]
        vt = [[resid.tile([P, P], BF16, tag=f"v{i}_{b}", name=f"v{i}_{b}") for b in range(NB)]
              for i in range(NKV)]

        z3 = [resid.tile([P, 512], BF16, tag=f"z3_{h}", name=f"z3_{h}")
              for h in range(NH)]
        # wo staging (bf16); loads issued near the end of attention so the
        # output projection never waits on them
        wostg = stk.enter_context(tc.tile_pool(name="wostg", bufs=1))
        stg = [wostg.tile([P, E], BF16, tag=f"wostg{h}", name=f"stg{h}")
               for h in range(NH)]
        zinbpool = stk.enter_context(tc.tile_pool(name="zinb", bufs=16))

        def load_z(qsb):
            if qsb == 3:
                return z3
            zin = []
            for h in range(NH):
                zb = zinbpool.tile([P, 512], BF16, tag="zinb")
                nc.sync.dma_start(
                    out=zb[:],
                    in_=zspill[h * P:(h + 1) * P, qsb * 512:(qsb + 1) * 512])
                zin.append(zb)
            return zin
        const = stk.enter_context(tc.tile_pool(name="const", bufs=1))
        # combined boundary masks, {BIG where allowed, 0 where disallowed},
        # each covering a 2-keyblock pair; applied as elementwise MIN: exp
        # output is >= 0 and at worst +inf (stale PSUM in skipped regions),
        # so min() zeroes disallowed slots NaN-free at full DVE speed.
        mdiag = const.tile([P, 512], BF16, tag="mdiag")
        mfar = const.tile([P, 512], BF16, tag="mfar")
        ones_f = const.tile([P, P], F32, tag="ones_f")
        ones = const.tile([P, P], BF16, tag="ones")

        def rope_evict(dest, psum, cos_sl, sin_sl, tmp_pool, n):
            # dest = psum * cos + pairswap(psum) * sin   (sin pre-signed)
            tmp = tmp_pool.tile([P, 512], F32, tag="ropetmp", name="ropetmp")
            nc.vector.stream_shuffle(tmp[:, :n], psum, SHUF_SWAP)
            nc.vector.tensor_mul(tmp[:, :n], tmp[:, :n], sin_sl)
            nc.vector.tensor_mul(dest, psum, cos_sl)
            nc.vector.tensor_add(dest, dest, tmp[:, :n])

        osb = stk.enter_context(tc.tile_pool(name="osb", bufs=8))
        with tc.tile_pool(name="pacc", bufs=5, space="PSUM") as pacc, \
             tc.tile_pool(name="pz", bufs=2, space="PSUM") as pzp, \
             tc.tile_pool(name="pb", bufs=1, space="PSUM") as pbp, \
             tc.tile_pool(name="quarter", bufs=2) as qpool, \
             tc.tile_pool(name="wq", bufs=3) as wqpool, \
             tc.tile_pool(name="work", bufs=4) as work, \
             tc.tile_pool(name="qt", bufs=2) as qtpool, \
             tc.tile_pool(name="zev", bufs=3) as zevpool, \
             tc.tile_pool(name="scr", bufs=2) as scrpool, \
             tc.tile_pool(name="rtmp", bufs=2) as rtmp:

            def load_quarter(qtr):
                xtc = [qpool.tile([P, 2048], BF16, tag=f"xt{dc}", name=f"xt{dc}")
                       for dc in range(4)]
                cos_q = qpool.tile([P, 512], F32, tag="cos")
                sin_q = qpool.tile([P, 512], F32, tag="sin")
                for dc in range(4):
                    nc.sync.dma_start(
                        out=xtc[dc][:],
                        in_=xq[qtr * P:(qtr + 1) * P, dc * 2048:(dc + 1) * 2048])
                c0 = 512 * qtr
                nc.sync.dma_start(out=cos_q[:], in_=cosT[:, c0:c0 + 512])
                nc.sync.dma_start(out=sin_q[:], in_=sinT[:, c0:c0 + 512])
                return xtc, cos_q, sin_q

            # startup: interleave kvw/x chunks in consumption order so the
            # first K-proj matmuls start after ~2MB instead of ~8MB
            xtc0 = [qpool.tile([P, 2048], BF16, tag=f"xt{dc}", name=f"xt{dc}")
                    for dc in range(4)]
            cos_q0 = qpool.tile([P, 512], F32, tag="cos")
            sin_q0 = qpool.tile([P, 512], F32, tag="sin")
            # K-half weights + x chunks first (the only startup-critical 3MB);
            # V-half weights and tables follow
            for dc in range(4):
                nc.sync.dma_start(out=kvk[dc][:],
                                  in_=wkv[:, dc * 1024:(dc + 1) * 1024])
                nc.sync.dma_start(out=xtc0[dc][:],
                                  in_=xq[:P, dc * 2048:(dc + 1) * 2048])
                if dc == 0:
                    nc.sync.dma_start(out=cos_q0[:], in_=cosT[:, :512])
                    nc.sync.dma_start(out=sin_q0[:], in_=sinT[:, :512])
            for dc in range(4):
                nc.sync.dma_start(out=kvv[dc][:],
                                  in_=wkv[:, 4096 + dc * 1024:4096 + (dc + 1) * 1024])
            cur = (xtc0, cos_q0, sin_q0)

            # rolling Wq prefetch, 3 heads deep (global head index)
            wq_q = []

            def wq_prefetch(g):
                if g >= 4 * NH:
                    return
                h = g % NH
                wqt = wqpool.tile([P, NKT * DH], BF16, tag="wqh", name="wqt")
                nc.sync.dma_start(out=wqt[:], in_=wqp[h * P:(h + 1) * P, :])
                wq_q.append(wqt)

            nc.sync.dma_start(out=mdiag[:], in_=masks[0:P, :])
            nc.sync.dma_start(out=mfar[:], in_=masks[P:2 * P, :])
            nc.vector.memset(ones_f[:], 1.0)
            nc.vector.tensor_copy(ones[:], ones_f[:])

            # lag-2 pipeline of attention tails
            pend = []

            def denom_tree(pt, nkb):
                # softmax denominator part 1: fold the key-block dim on DVE
                # (tree of contiguous pairwise adds; dead boundary halves are
                # zeroed by the masks). Issued right after the masks so the
                # DVE work is long done when the tail's pb matmul needs it.
                ksum = zevpool.tile([P, 256], BF16, tag="ksum")

                def fl(tile_, a, b):
                    return tile_[:, a:b, :].rearrange("p k q -> p (k q)")

                if nkb == 2:
                    nc.vector.tensor_add(ksum[:], pt[:, 0, :], pt[:, 1, :])
                    return ksum
                scr = scrpool.tile([P, 5, 256], BF16, tag="scr")
                hb = nkb // 2
                nc.vector.tensor_add(fl(scr, 0, hb), fl(pt, 0, hb), fl(pt, hb, nkb))
                if nkb == 4:
                    nc.vector.tensor_add(ksum[:], scr[:, 0, :], scr[:, 1, :])
                elif nkb == 6:
                    nc.vector.tensor_add(scr[:, 0, :], scr[:, 0, :], scr[:, 1, :])
                    nc.vector.tensor_add(ksum[:], scr[:, 0, :], scr[:, 2, :])
                elif nkb == 8:
                    nc.vector.tensor_add(fl(scr, 0, 2), fl(scr, 0, 2), fl(scr, 2, 4))
                    nc.vector.tensor_add(ksum[:], scr[:, 0, :], scr[:, 1, :])
                else:  # nkb == 10
                    nc.vector.tensor_add(fl(scr, 0, 2), fl(scr, 0, 2), fl(scr, 2, 4))
                    nc.vector.tensor_add(scr[:, 0, :], scr[:, 0, :], scr[:, 1, :])
                    nc.vector.tensor_add(ksum[:], scr[:, 0, :], scr[:, 4, :])
                return ksum

            def attn_tail():
                kv, kbs, pt, h, t, ksum = pend.pop(0)
                nkb = len(kbs)

                def qspan(kb):
                    # diagB (kb==2t+1) is all-masked for the first 128 queries
                    # of the super; farA (kb==2t-8) for the last 128. Skip the
                    # dead half in the PV matmuls (partial-width PSUM
                    # accumulation zero-fills untouched columns).
                    if kb == 2 * t + 1:
                        return 128, 256
                    if kb == 2 * t - 8:
                        return 0, 128
                    return 0, 256

                # PV
                pz = pzp.tile([P, 256], F32, tag="pz")
                for i, kb in enumerate(kbs):
                    a, b = qspan(kb)
                    nc.tensor.matmul(
                        pz[:, a:b], vt[kv][kb][:],
                        pt[:, i, a:b],
                        start=(i == 0), stop=(i == nkb - 1))
                # denominator part 2: one ones-matmul broadcasts the
                # cross-partition sum into PSUM
                pb = pbp.tile([P, 256], F32, tag="pb")
                nc.tensor.matmul(pb[:], ones[:], ksum[:], start=True, stop=True)
                rec = zevpool.tile([P, 256], F32, tag="rec")
                nc.vector.reciprocal_approx_fast(rec[:], pb[:])
                if t >= 6:
                    s3 = t - 6
                    nc.vector.tensor_mul(
                        z3[h][:, s3 * 256:(s3 + 1) * 256], pz[:], rec[:])
                else:
                    zev = zevpool.tile([P, 256], BF16, tag="zev")
                    nc.vector.tensor_mul(zev[:], pz[:], rec[:])
                    nc.sync.dma_start(
                        out=zspill[h * P:(h + 1) * P, t * 256:(t + 1) * 256],
                        in_=zev[:])

            for qtr in range(4):
                xt, cos_q, sin_q = cur

                def xtile(kt, a, b):
                    return xt[kt // 4][:, (kt % 4) * 512 + a: (kt % 4) * 512 + b]

                def kvw_sl(kt, a, b):
                    # K-half (a,b in [0,256)) from kvk, V-half from kvv
                    if b <= 256:
                        return kvk[kt // 4][:, (kt % 4) * 256 + a: (kt % 4) * 256 + b]
                    return kvv[kt // 4][:, (kt % 4) * 256 + a - 256: (kt % 4) * 256 + b - 256]

                c0 = 512 * qtr

                # K projection (+RoPE) for both kv heads
                for kv in range(NKV):
                    pk = pacc.tile([P, 512], F32, tag="pacc")
                    for kt in range(NKT):
                        nc.tensor.matmul(
                            pk[:],
                            kvw_sl(kt, kv * DH, (kv + 1) * DH),
                            xtile(kt, 0, 512),
                            start=(kt == 0), stop=(kt == NKT - 1),
                        )
                    rope_evict(kT[kv][:, c0:c0 + 512], pk[:], cos_q[:], sin_q[:], rtmp, 512)

                if qtr == 0:
                    # deferred so these don't share DMA bandwidth with the
                    # critical kvw+xt startup burst
                    for g0 in range(3):
                        wq_prefetch(g0)

                # V projection (both kv heads at once, natural layout)
                for lb in range(4):
                    pv = pacc.tile([P, 512], F32, tag="pacc")
                    for kt in range(NKT):
                        nc.tensor.matmul(
                            pv[:, :NKV * DH],
                            xtile(kt, lb * P, (lb + 1) * P),
                            kvw_sl(kt, 256, 512),
                            start=(kt == 0), stop=(kt == NKT - 1),
                        )
                    for kv in range(NKV):
                        nc.scalar.copy(vt[kv][4 * qtr + lb][:], pv[:, kv * DH:(kv + 1) * DH])

                def q_proj(g):
                    """Q projection + RoPE for one head; returns the roped tile."""
                    wq = wq_q.pop(0)
                    wq_prefetch(g + 3)
                    pq = pacc.tile([P, 512], F32, tag="pacc")
                    for kt in range(NKT):
                        nc.tensor.matmul(
                            pq[:],
                            wq[:, kt * DH:(kt + 1) * DH],
                            xtile(kt, 0, 512),
                            start=(kt == 0), stop=(kt == NKT - 1),
                        )
                    qth = qtpool.tile([P, 512], BF16, tag="qt")
                    rope_evict(qth[:], pq[:], cos_q[:], sin_q[:], rtmp, 512)
                    return qth

                # Q projection + attention, head-major; Q runs one head ahead
                # so RoPE latency is always covered by PE work
                qnext = q_proj(qtr * NH)
                for h in range(NH):
                    kv = h // (NH // NKV)
                    qth = qnext
                    if h + 1 < NH:
                        qnext = q_proj(qtr * NH + h + 1)
                    if h == 1 and qtr < 3:
                        cur = load_quarter(qtr + 1)
                    if h == 1 and qtr == 3:
                        # prefetch wo for the output projection; by now the
                        # DMA engines are mostly idle
                        for hh in range(NH):
                            nc.sync.dma_start(out=stg[hh][:],
                                              in_=woT[hh * P:(hh + 1) * P, :])
                    if h == 4 and qtr == 3:
                        zpre0 = load_z(0)
                    if h == 6 and qtr == 3:
                        zpre1 = load_z(1)
                    for s in range(2):
                        t = 2 * qtr + s
                        qt = qth[:, s * 256:(s + 1) * 256]

                        # drain the oldest pending super (lag 2)
                        if len(pend) >= 2:
                            attn_tail()

                        kbs = _kbs_for_super(t)
                        nkb = len(kbs)
                        pt = work.tile([P, 10, 256], BF16, tag="pt")
                        # scores (transposed: keys on partitions) in chunks of
                        # 2 kb; dead query-halves of the boundary blocks are
                        # skipped (untouched PSUM reads back as zero, and the
                        # full-width masks zero those slots after exp)
                        for ci in range(0, nkb, 2):
                            cn = min(2, nkb - ci)
                            ps = pacc.tile([P, 512], F32, tag="pacc")
                            for i in range(cn):
                                kb = kbs[ci + i]
                                if kb == 2 * t + 1:
                                    a, b = 128, 256
                                elif kb == 2 * t - 8:
                                    a, b = 0, 128
                                else:
                                    a, b = 0, 256
                                nc.tensor.matmul(
                                    ps[:, i * 256 + a:i * 256 + b],
                                    kT[kv][:, kb * P:(kb + 1) * P],
                                    qt[:, a:b],
                                    start=True, stop=True,
                                )
                            nc.scalar.activation(
                                pt[:, ci:ci + cn, :].rearrange("p k q -> p (k q)"),
                                ps[:, :cn * 256], EXP, scale=SCALE)
                        # combined window masks on the boundary pairs (the
                        # diag pair is always the last two blocks; the far
                        # pair exists only for t>=4 and is the first two)
                        dsl = pt[:, nkb - 2:nkb, :].rearrange("p k q -> p (k q)")
                        nc.vector.tensor_tensor(dsl, dsl, mdiag[:],
                                                op=mybir.AluOpType.min)
                        if t >= 4:
                            fsl = pt[:, 0:2, :].rearrange("p k q -> p (k q)")
                            nc.vector.tensor_tensor(fsl, fsl, mfar[:],
                                                    op=mybir.AluOpType.min)
                        ksum = denom_tree(pt, nkb)
                        pend.append((kv, kbs, pt, h, t, ksum))

            while pend:
                attn_tail()

        # Output projection: out[q,:] += sum_h zTn_h[:,q].T @ woT[h]
        # wo and z are bf16 and feed the matmuls directly (bf16 matmul runs
        # at the same rate as f32r; the data was bf16-quantized anyway).
        with tc.tile_pool(name="po", bufs=8, space="PSUM") as pop:
            zs = {0: zpre0, 1: zpre1}
            for qsb in range(4):
                zin = zs.pop(qsb) if qsb in zs else load_z(qsb)
                for ec in range(4):
                    po = [pop.tile([P, 512], F32, tag="po", name=f"po{i}")
                          for i in range(4)]
                    for h in range(NH):
                        for qb in range(4):
                            nc.tensor.matmul(
                                po[qb][:],
                                zin[h][:, qb * P:(qb + 1) * P],
                                stg[h][:, ec * 512:(ec + 1) * 512],
                                start=(h == 0), stop=(h == NH - 1),
                            )
                    for qb in range(4):
                        ot = osb.tile([P, 512], BF16, tag="ot")
                        # alternate copy engine so the final evictions drain 2x
                        if qb % 2 == 0:
                            nc.scalar.copy(ot[:], po[qb][:])
                        else:
                            nc.vector.tensor_copy(ot[:], po[qb][:])
                        nc.sync.dma_start(
                            out=out[qsb * 512 + qb * P: qsb * 512 + (qb + 1) * P,
                                    ec * 512:(ec + 1) * 512],
                            in_=ot[:])
                    if qsb == 0 and ec == 3:
                        # prefetch qsb2 (reuses qsb0's buffers; issued after
                        # all qsb0 matmuls so the WAR ordering is clean)
                        zs[2] = load_z(2)

    nc.compile()
    return nc


def _host_tables():
    freqs = 1.0 / (THETA ** (np.arange(0, DH - 1, 2, dtype=np.float64) / DH))
    ang = np.arange(L, dtype=np.float64)[:, None] * freqs[None, :]  # (L, 64)
    cos = np.cos(ang)
    sin = np.sin(ang)
    cosT = np.empty((P, L), np.float32)
    sinT = np.empty((P, L), np.float32)
    cosT[0::2, :] = cos.T
    cosT[1::2, :] = cos.T
    sinT[0::2, :] = -sin.T
    sinT[1::2, :] = sin.T
    return cosT, sinT


def _host_masks():
    # {BIG where allowed, 0 where disallowed}; applied as elementwise min
    k = np.arange(P)[:, None]
    q = np.arange(256)[None, :]
    import ml_dtypes
    mdiag = np.concatenate([(k <= q), (k <= q - 128)], axis=1)   # (128, 512)
    mfar = np.concatenate([(k >= q + 1), (k >= q - 127)], axis=1)
    m = np.concatenate([mdiag, mfar], axis=0).astype(np.float32) * 3.0e38
    return m.astype(ml_dtypes.bfloat16)


def _pack_core_inputs(x, Wq, Wk, Wv, Wo, n, g):
    """Prepacked per-core inputs; long contiguous per-partition DMA runs."""
    xT = np.ascontiguousarray(x[n].T)                      # (E, L)
    # xq[qtr*128+p, kt*512+c] = xT[kt*128+p, qtr*512+c]
    xq = xT.reshape(NKT, P, 4, 512).transpose(2, 1, 0, 3).reshape(4 * P, NKT * 512)
    # wqp[h*128+p, kt*128+c] = Wq.T[kt*128+p, g*1024+h*128+c]
    wqT = Wq[g * 1024:(g + 1) * 1024, :].T                 # (E, 1024)
    wqp = wqT.reshape(NKT, P, NH, DH).transpose(2, 1, 0, 3).reshape(NH * P, NKT * DH)
    xq = np.ascontiguousarray(xq)
    wqp = np.ascontiguousarray(wqp)
    # wkv[p, kt*512+j]: j<256 -> Wk.T slice, j>=256 -> Wv.T slice
    wkT = Wk[g * 256:(g + 1) * 256, :].T.reshape(NKT, P, 256)
    wvT = Wv[g * 256:(g + 1) * 256, :].T.reshape(NKT, P, 256)
    # K-half first, V-half second (V is not startup-critical)
    wkvp = np.concatenate([
        wkT.transpose(1, 0, 2).reshape(P, NKT * 256),
        wvT.transpose(1, 0, 2).reshape(P, NKT * 256)], axis=1)
    woT = Wo[:, g * 1024:(g + 1) * 1024].T                 # (1024, E)
    import ml_dtypes
    return {
        "xq": xq.astype(ml_dtypes.bfloat16),
        "wqp": wqp.astype(ml_dtypes.bfloat16),
        "wkv": np.ascontiguousarray(wkvp).astype(ml_dtypes.bfloat16),
        "woT": np.ascontiguousarray(woT).astype(ml_dtypes.bfloat16),
    }


def kernel(x, Wq, Wk, Wv, Wo):
    global _NC
    x = np.asarray(x, np.float32)
    Wq = np.asarray(Wq, np.float32)
    Wk = np.asarray(Wk, np.float32)
    Wv = np.asarray(Wv, np.float32)
    Wo = np.asarray(Wo, np.float32)

    if _NC is None:
        _NC = build_nc()
    nc = _NC

    cosT, sinT = _host_tables()
    masks = _host_masks()
    in_maps = []
    for c in range(8):
        n, g = c % 4, c // 4
        m = _pack_core_inputs(x, Wq, Wk, Wv, Wo, n, g)
        m.update({"cosT": cosT, "sinT": sinT, "masks": masks})
        in_maps.append(m)

    from concourse.bass_utils import run_bass_kernel_spmd
    res = run_bass_kernel_spmd(nc, in_maps, list(range(8)), trace=False)
    out = np.empty((N, L, E), np.float32)
    for n_ in range(4):
        out[n_] = (res.results[n_]["out"].astype(np.float32)
                   + res.results[4 + n_]["out"].astype(np.float32))
    return out


if __name__ == "__main__":
    rng = np.random.default_rng(0)
    x = rng.standard_normal((N, L, E), dtype=np.float32)
    Wq = (rng.standard_normal((E, E), dtype=np.float32) * 0.02)
    Wk = (rng.standard_normal((E // D, E), dtype=np.float32) * 0.02)
    Wv = (rng.standard_normal((E // D, E), dtype=np.float32) * 0.02)
    Wo = (rng.standard_normal((E, E), dtype=np.float32) * 0.02)
    print(kernel(x, Wq, Wk, Wv, Wo).shape)

